# revision 8
# baseline (speedup 1.0000x reference)
"""DiT forward on 8 Trainium2 NeuronCores, data-parallel over batch.

Per-core program (4 images, T=1024 tokens): activations kept feature-major
(features on SBUF partitions, tokens on the free axis) so that
- every linear runs as matmul(lhsT=W_block, rhs=act) with W in natural layout,
- adaLN modulate scale/bias are per-partition tensor_scalar operands,
- LN token-stats come from an all-ones lhsT matmul (partition-broadcast sums).
The residual stream, stem and LN statistics run in float32r (full PE rate,
~1e-4 matmul rel err); the branch matmuls (qkv/attention/proj/mlp/cond) run
in bf16 to fit SBUF — their outputs are small gated residual updates.
The attention v-bias is folded into proj_b host-side (softmax rows sum to 1).
"""
import sys

sys.path.insert(0, "/opt/trn_rl_repo")

import numpy as np
import ml_dtypes

import concourse.bass as bass
import concourse.mybir as mybir
from concourse import bacc
from concourse.tile import TileContext
from concourse.bass_utils import run_bass_kernel_spmd

P = 128
B, C_IN, IMG, PS = 32, 4, 256, 16
GRID = IMG // PS          # 16
L = GRID * GRID           # 256 tokens per image
D = 1024
KT = D // P               # 8
NH, HD = 16, 64
NL = 6
NCORES = 8
IPC = B // NCORES         # 4 images per core
T = IPC * L               # 1024 tokens per core
NS = 2                    # token slices of 512
SL = T // NS              # 512
MAX_L = 10000.0
LN_EPS = 1e-5

F32 = mybir.dt.float32
F32R = mybir.dt.float32r
BF16 = mybir.dt.bfloat16
AF = mybir.ActivationFunctionType
ALU = mybir.AluOpType


def _bias_cols():
    """Column map for the packed per-partition bias tile [128, NB]."""
    cols = {}
    c = 0

    def add(name, n):
        nonlocal c
        cols[name] = c
        c += n

    add("eps", 1)
    add("xe", 8)
    add("tb1", 8)
    add("tb2", 8)
    add("fin", 16)
    for l in range(NL):
        add(f"qk{l}", 16)
        add(f"proj{l}", 8)
        add(f"b1{l}", 32)
        add(f"b2{l}", 8)
        add(f"cond{l}", 48)
    return cols, c


BCOLS, NB = _bias_cols()


# ---------------------------------------------------------------- device build
def _build():
    nc = bacc.Bacc("TRN2", target_bir_lowering=False, debug=False)

    def dram(name, shape, dt, out=False):
        return nc.dram_tensor(name, list(shape), dt,
                              kind="ExternalOutput" if out else "ExternalInput").ap()

    xT0 = dram("xT0", [D, T], F32R)
    te_d = dram("te", [D, IPC], F32R)
    yemb_d = dram("yemb", [D, IPC], F32R)
    xe_w = dram("xe_w", [8, P, KT, P], F32R)
    tw1 = dram("tw1", [8, P, KT, P], F32R)
    tw2 = dram("tw2", [8, P, KT, P], F32R)
    fin_w = dram("fin_w", [16, P, KT, P], F32R)
    qk_w = dram("qk_w", [NL, 16, P, KT, P], BF16)
    v_w = dram("v_w", [NL, KT, P, D], BF16)
    proj_w = dram("proj_w", [NL, 8, P, KT, P], BF16)
    m1_w = dram("m1_w", [NL, 32, P, KT, P], BF16)
    m2_w = dram("m2_w", [NL, 2, 8, P, 16, P], BF16)
    cond_w = dram("cond_w", [NL, 48, P, KT, P], BF16)
    bias_pp = dram("bias_pp", [P, NB], F32)
    onesr_d = dram("onesr", [P, P], F32R)
    onesb_d = dram("onesb", [P, P], BF16)
    outT = dram("outT", [D, T], F32, out=True)

    from contextlib import ExitStack
    with TileContext(nc) as tc, ExitStack() as ctx:
        cst = ctx.enter_context(tc.tile_pool(name="cst", bufs=1))
        pA = ctx.enter_context(tc.tile_pool(name="pA", bufs=1))
        pB = ctx.enter_context(tc.tile_pool(name="pB", bufs=1))
        pD = ctx.enter_context(tc.tile_pool(name="pD", bufs=1))
        wp = ctx.enter_context(tc.tile_pool(name="wp", bufs=2))
        ep = ctx.enter_context(tc.tile_pool(name="ep", bufs=2))
        st = ctx.enter_context(tc.tile_pool(name="st", bufs=1))
        psp = ctx.enter_context(tc.tile_pool(name="psp", bufs=4, space="PSUM"))
        psS = ctx.enter_context(tc.tile_pool(name="psS", bufs=2, space="PSUM"))
        psO = ctx.enter_context(tc.tile_pool(name="psO", bufs=1, space="PSUM"))

        ones_r = cst.tile([P, P], F32R)
        nc.sync.dma_start(ones_r, onesr_d)
        ones_b = cst.tile([P, P], BF16)
        nc.sync.dma_start(ones_b, onesb_d)
        bpp = cst.tile([P, NB], F32)
        nc.sync.dma_start(bpp, bias_pp)

        def bcol(name, mt=0):
            return bpp[:, BCOLS[name] + mt: BCOLS[name] + mt + 1]

        # ---- conditioning path (tiny, N=4) ----
        # cpk groups: 0=te 1=yemb 2=u1 3=cond  (all f32r)
        cpk = cst.tile([P, KT, 4, IPC], F32R)
        nc.sync.dma_start(cpk[:, :, 0], te_d.rearrange("(kt p) i -> p kt i", p=P))
        nc.sync.dma_start(cpk[:, :, 1], yemb_d.rearrange("(kt p) i -> p kt i", p=P))
        scond = cst.tile([P, KT, IPC], BF16)

        def small_fm(wdram, n_mt, act_grp, epilogue, wtag="w8", dt_=None):
            for mt in range(n_mt):
                wt = wp.tile([P, KT, P], dt_ or F32R, tag=wtag)
                nc.sync.dma_start(wt, wdram[mt])
                ps = psp.tile([P, IPC], F32, tag="ps512")
                for kt in range(KT):
                    nc.tensor.matmul(ps, wt[:, kt], act_grp(kt),
                                     start=kt == 0, stop=kt == KT - 1)
                epilogue(ps, mt)

        small_fm(tw1, 8, lambda kt: cpk[:, kt, 0],
                 lambda ps, mt: nc.scalar.activation(
                     cpk[:, mt, 2], ps, AF.Silu, bias=bcol("tb1", mt)))

        def temb_ep(ps, mt):
            tmp = ep.tile([P, SL], F32, tag="ptmp")
            nc.scalar.activation(tmp[:, :IPC], ps, AF.Identity, bias=bcol("tb2", mt))
            nc.vector.tensor_add(cpk[:, mt, 3], tmp[:, :IPC], cpk[:, mt, 1])
            nc.scalar.activation(scond[:, mt], cpk[:, mt, 3], AF.Silu)

        small_fm(tw2, 8, lambda kt: cpk[:, kt, 2], temb_ep)

        # mods for all layers + final, packed: [P, 6*48+16, IPC] f32
        mpk = cst.tile([P, NL * 48 + 16, IPC], F32)
        for l in range(NL):
            small_fm(cond_w[l], 48, lambda kt: scond[:, kt],
                     lambda ps, mt, l=l: nc.scalar.activation(
                         mpk[:, l * 48 + mt], ps, AF.Identity,
                         bias=bcol(f"cond{l}", mt)),
                     wtag="w8b", dt_=BF16)
        small_fm(fin_w, 16, lambda kt: cpk[:, kt, 3],
                 lambda ps, mt: nc.scalar.activation(
                     mpk[:, NL * 48 + mt], ps, AF.Identity, bias=bcol("fin", mt)))

        # ---- stem: h = patchify(x) @ x_embed_w + b (feature-major) ----
        hT = cst.tile([P, KT, T], F32R)
        for s in range(NS):
            xsb = pA.tile([P, KT, SL], F32R, tag="slotA")
            for kt in range(KT):
                nc.sync.dma_start(xsb[:, kt],
                                  xT0[kt * P:(kt + 1) * P, s * SL:(s + 1) * SL])
            for mt in range(KT):
                wt = wp.tile([P, KT, P], F32R, tag="w8")
                nc.sync.dma_start(wt, xe_w[mt])
                ps = psp.tile([P, SL], F32, tag="ps512")
                for kt in range(KT):
                    nc.tensor.matmul(ps, wt[:, kt], xsb[:, kt],
                                     start=kt == 0, stop=kt == KT - 1)
                nc.scalar.activation(hT[:, mt, s * SL:(s + 1) * SL], ps,
                                     AF.Identity, bias=bcol("xe", mt))

        def linear_fm(wdram, n_mt, act, epilogue, dt_=BF16, wtag="w8b"):
            # out[mt,s] = sum_kt W[kt,mt].T @ act[kt,s]
            for mt in range(n_mt):
                wt = wp.tile([P, KT, P], dt_, tag=wtag)
                nc.sync.dma_start(wt, wdram[mt])
                for s in range(NS):
                    ps = psp.tile([P, SL], F32, tag="ps512")
                    for kt in range(KT):
                        nc.tensor.matmul(ps, wt[:, kt],
                                         act[:, kt, s * SL:(s + 1) * SL],
                                         start=kt == 0, stop=kt == KT - 1)
                    epilogue(ps, mt, s)

        # ---- LN + modulate (feature-major; stats via all-ones matmul) ----
        def ln_mod(src, s_col, b_col, out_t=None, final=False):
            """out = (src - mu) * rstd * mods[s_col] + mods[b_col] per image.
            final=True: stream [P,SL] pieces straight to outT DRAM."""
            for s in range(NS):
                sc = slice(s * SL, (s + 1) * SL)
                sq = pA.tile([P, KT, SL], F32R, tag="slotA")
                for kt in range(KT):
                    nc.vector.tensor_mul(sq[:, kt], src[:, kt, sc], src[:, kt, sc])
                ps_sum = psp.tile([P, SL], F32, tag="ps512")
                for kt in range(KT):
                    nc.tensor.matmul(ps_sum, ones_r, src[:, kt, sc],
                                     start=kt == 0, stop=kt == KT - 1)
                ps_sq = psp.tile([P, SL], F32, tag="ps512")
                for kt in range(KT):
                    nc.tensor.matmul(ps_sq, ones_r, sq[:, kt],
                                     start=kt == 0, stop=kt == KT - 1)
                # stats pack: 0=mu 1=var 2=musq 3=rstd
                sp = st.tile([P, 4, SL], F32, tag="stats")
                nc.vector.tensor_scalar_mul(sp[:, 0], ps_sum, 1.0 / D)
                nc.vector.tensor_scalar_mul(sp[:, 1], ps_sq, 1.0 / D)
                nc.vector.tensor_mul(sp[:, 2], sp[:, 0], sp[:, 0])
                nc.vector.tensor_tensor(sp[:, 1], sp[:, 1], sp[:, 2], ALU.subtract)
                nc.scalar.activation(sp[:, 1], sp[:, 1], AF.Sqrt, bias=bcol("eps"))
                nc.vector.reciprocal(sp[:, 3], sp[:, 1])
                for kt in range(KT):
                    t_ = ep.tile([P, SL], F32, tag="lnt")
                    nc.vector.tensor_tensor(t_, src[:, kt, sc], sp[:, 0],
                                            ALU.subtract)
                    nc.vector.tensor_tensor(t_, t_, sp[:, 3], ALU.mult)
                    if final:
                        o = ep.tile([P, SL], F32, tag="ptmp", name="ofin")
                    else:
                        o = None
                    for i2 in range(2):
                        img = 2 * s + i2
                        i2c = slice(i2 * L, (i2 + 1) * L)
                        dst = o[:, i2c] if final else out_t[:, kt, img * L:(img + 1) * L]
                        nc.vector.tensor_scalar(
                            dst, t_[:, i2c],
                            mpk[:, s_col + kt, img:img + 1],
                            mpk[:, b_col + kt, img:img + 1],
                            ALU.mult, ALU.add)
                    if final:
                        nc.gpsimd.dma_start(outT[kt * P:(kt + 1) * P, sc], o)

        # ---- transformer layers ----
        for l in range(NL):
            mb = l * 48
            # attention branch
            a1 = pB.tile([P, KT, T], BF16, tag="slotB")
            ln_mod(hT, mb + 0, mb + 8, a1)

            qkT = pD.tile([P, 16, T], BF16, tag="slotD")
            linear_fm(qk_w[l], 16, a1, lambda ps, mt, s, l=l: nc.scalar.activation(
                qkT[:, mt, s * SL:(s + 1) * SL], ps, AF.Identity,
                bias=bcol(f"qk{l}", mt)))

            vtok = pA.tile([P, KT, D], BF16, tag="slotA")
            for nh in range(4):
                vw = wp.tile([P, KT, 256], BF16, tag="vw")
                nc.sync.dma_start(
                    vw, v_w[l][:, :, nh * 256:(nh + 1) * 256].rearrange(
                        "kt p n -> p kt n"))
                for vt in range(KT):
                    ps = psp.tile([P, 256], F32, tag="ps512")
                    for kt in range(KT):
                        nc.tensor.matmul(ps, a1[:, kt, vt * P:(vt + 1) * P],
                                         vw[:, kt], start=kt == 0,
                                         stop=kt == KT - 1)
                    nc.scalar.activation(vtok[:, vt, nh * 256:(nh + 1) * 256],
                                         ps, AF.Copy)

            attnT = pB.tile([P, KT, T], BF16, tag="slotB")
            for img in range(IPC):
                ic = slice(img * L, (img + 1) * L)
                for h in range(NH):
                    off = 64 * (h % 2)
                    pt = h // 2
                    qs = qkT[off:off + 64, pt, ic]
                    expS = ep.tile([P, 2, L], BF16, tag="expS")
                    for jt in range(2):
                        pss = psS.tile([P, L], F32, tag="psS")
                        j0 = img * L + jt * P
                        ks = qkT[off:off + 64, 8 + pt, j0:j0 + P]
                        nc.tensor.matmul(pss, ks, qs, start=True, stop=True)
                        nc.scalar.activation(expS[:, jt], pss, AF.Exp,
                                             scale=float(HD) ** -0.5)
                    psd = psO.tile([P, L], F32, tag="psD")
                    for jt in range(2):
                        nc.tensor.matmul(psd, ones_b, expS[:, jt],
                                         start=jt == 0, stop=jt == 1)
                    psv = psO.tile([64, L], F32, tag="psV")
                    for jt in range(2):
                        nc.tensor.matmul(psv,
                                         vtok[:, img * 2 + jt, h * 64:(h + 1) * 64],
                                         expS[:, jt], start=jt == 0, stop=jt == 1)
                    rec = ep.tile([64, L], F32, tag="rec")
                    nc.vector.reciprocal(rec, psd[0:64, :])
                    nc.vector.tensor_tensor(attnT[off:off + 64, pt, ic],
                                            psv, rec, ALU.mult)

            def proj_ep(ps, mt, s, l=l, mb=mb):
                sc = slice(s * SL, (s + 1) * SL)
                tmp = ep.tile([P, SL], F32, tag="ptmp")
                nc.scalar.activation(tmp, ps, AF.Identity, bias=bcol(f"proj{l}", mt))
                for i2 in range(2):
                    img = 2 * s + i2
                    nc.vector.tensor_scalar_mul(
                        tmp[:, i2 * L:(i2 + 1) * L], tmp[:, i2 * L:(i2 + 1) * L],
                        mpk[:, mb + 16 + mt, img:img + 1])
                nc.vector.tensor_tensor(hT[:, mt, sc], hT[:, mt, sc], tmp, ALU.add)

            linear_fm(proj_w[l], 8, attnT, proj_ep)

            # mlp branch
            a2 = pB.tile([P, KT, T], BF16, tag="slotB")
            ln_mod(hT, mb + 24, mb + 32, a2)
            macc = pA.tile([P, KT, T], BF16, tag="slotA")
            h1 = pD.tile([P, 16, T], BF16, tag="slotD")
            for hf in range(2):
                linear_fm(m1_w[l][hf * 16:(hf + 1) * 16], 16, a2,
                          lambda ps, mt, s, hf=hf, l=l: nc.scalar.activation(
                              h1[:, mt, s * SL:(s + 1) * SL], ps, AF.Silu,
                              bias=bcol(f"b1{l}", hf * 16 + mt)))
                for mt in range(KT):
                    w2t = wp.tile([P, 16, P], BF16, tag="w16")
                    nc.sync.dma_start(w2t, m2_w[l, hf, mt])
                    for s in range(NS):
                        sc = slice(s * SL, (s + 1) * SL)
                        ps = psp.tile([P, SL], F32, tag="ps512")
                        for kt2 in range(16):
                            nc.tensor.matmul(ps, w2t[:, kt2], h1[:, kt2, sc],
                                             start=kt2 == 0, stop=kt2 == 15)
                        if hf == 0:
                            nc.vector.tensor_copy(macc[:, mt, sc], ps)
                        else:
                            tmp = ep.tile([P, SL], F32, tag="ptmp")
                            nc.vector.tensor_tensor(tmp, macc[:, mt, sc], ps,
                                                    ALU.add)
                            for i2 in range(2):
                                img = 2 * s + i2
                                i2c = slice(i2 * L, (i2 + 1) * L)
                                nc.vector.tensor_scalar(
                                    tmp[:, i2c], tmp[:, i2c],
                                    bcol(f"b2{l}", mt),
                                    mpk[:, mb + 40 + mt, img:img + 1],
                                    ALU.add, ALU.mult)
                            nc.vector.tensor_tensor(hT[:, mt, sc],
                                                    hT[:, mt, sc], tmp, ALU.add)

        # ---- final LN + modulate -> outT ----
        ln_mod(hT, NL * 48 + 0, NL * 48 + 8, final=True)

    nc.compile()
    return nc


_NC_CACHE = None


def _get_nc():
    global _NC_CACHE
    if _NC_CACHE is None:
        _NC_CACHE = _build()
    return _NC_CACHE


# ---------------------------------------------------------------- host side
def _blocks(w, dtype=np.float32):
    """[K, M] -> [MT, 128(k), KT, 128(m)] so each mt slice is one DMA."""
    K, M = w.shape
    return np.ascontiguousarray(
        w.reshape(K // P, P, M // P, P).transpose(2, 1, 0, 3)).astype(dtype)


def _pack_biases(inp, proj_b_eff):
    bp = np.zeros((P, NB), np.float32)

    def put(name, b, bake1=()):
        b = np.asarray(b, np.float32).copy()
        for lo, hi in bake1:
            b[lo:hi] += 1.0
        n = b.shape[0] // P
        bp[:, BCOLS[name]:BCOLS[name] + n] = b.reshape(n, P).T

    bp[:, BCOLS["eps"]] = LN_EPS
    put("xe", inp["x_embed_b"])
    put("tb1", inp["t_b1"])
    put("tb2", inp["t_b2"])
    put("fin", inp["final_b"], bake1=[(0, D)])
    for l in range(NL):
        put(f"qk{l}", inp["qkv_b"][l][:2 * D])
        put(f"proj{l}", proj_b_eff[l])
        put(f"b1{l}", inp["mlp_b1"][l])
        put(f"b2{l}", inp["mlp_b2"][l])
        put(f"cond{l}", inp["cond_b"][l], bake1=[(0, D), (3 * D, 4 * D)])
    return bp


def kernel(**inputs):
    inp = {k: np.asarray(v) for k, v in inputs.items()}
    nc = _get_nc()
    bf16 = ml_dtypes.bfloat16

    x = inp["x"].astype(np.float32)
    patch = x.reshape(B, C_IN, GRID, PS, GRID, PS).transpose(
        0, 2, 4, 1, 3, 5).reshape(B, L, C_IN * PS * PS)

    t = inp["t"].astype(np.float32)
    angles = MAX_L ** (-(np.arange(0, D, 2, dtype=np.float32) / D))
    te = t[:, None] * angles[None, :]
    te = np.concatenate([np.sin(te), np.cos(te)], axis=-1).astype(np.float32)
    yemb = inp["y_table"][inp["y"]].astype(np.float32)

    # fold v-bias into proj bias: softmax rows sum to 1 -> attn_out += v_bias
    vb = inp["qkv_b"][:, 2 * D:].astype(np.float32)                  # [NL, D]
    proj_b_eff = inp["proj_b"].astype(np.float32) + np.einsum(
        "ld,ldm->lm", vb, inp["proj_w"].astype(np.float32))

    shared = {
        "xe_w": _blocks(inp["x_embed_w"]),
        "tw1": _blocks(inp["t_w1"]),
        "tw2": _blocks(inp["t_w2"]),
        "fin_w": _blocks(inp["final_w"]),
        "qk_w": np.stack([_blocks(inp["qkv_w"][l][:, :2 * D], bf16)
                          for l in range(NL)]),
        "v_w": np.ascontiguousarray(inp["qkv_w"][:, :, 2 * D:]).reshape(
            NL, KT, P, D).astype(bf16),
        "proj_w": np.stack([_blocks(inp["proj_w"][l], bf16) for l in range(NL)]),
        "m1_w": np.stack([_blocks(inp["mlp_w1"][l], bf16) for l in range(NL)]),
        "m2_w": np.stack([
            inp["mlp_w2"][l].reshape(2, 16, P, KT, P).transpose(0, 3, 2, 1, 4)
            for l in range(NL)]).astype(bf16),
        "cond_w": np.stack([_blocks(inp["cond_w"][l], bf16) for l in range(NL)]),
        "bias_pp": _pack_biases(inp, proj_b_eff),
        "onesr": np.ones((P, P), np.float32),
        "onesb": np.ones((P, P), bf16),
    }
    in_maps = []
    for c in range(NCORES):
        sl = slice(c * IPC, (c + 1) * IPC)
        m = dict(shared)
        m["xT0"] = np.ascontiguousarray(
            patch[sl].reshape(T, D).T, dtype=np.float32)
        m["te"] = np.ascontiguousarray(te[sl].T)
        m["yemb"] = np.ascontiguousarray(yemb[sl].T)
        in_maps.append(m)

    res = run_bass_kernel_spmd(nc, in_maps, core_ids=list(range(NCORES)))

    outs = []
    for c in range(NCORES):
        oT = res.results[c]["outT"]                       # [D, T]
        o = oT.T.reshape(IPC, GRID, GRID, C_IN, PS, PS)   # tokens -> patches
        outs.append(o.transpose(0, 3, 1, 4, 2, 5).reshape(IPC, C_IN, IMG, IMG))
    return np.concatenate(outs, axis=0).astype(np.float32)


# revision 9
# speedup vs baseline: 351.8098x; 351.8098x over previous
"""DiT forward on 8 Trainium2 NeuronCores, data-parallel over batch.

Per-core program (4 images, T=1024 tokens): activations kept feature-major
(features on SBUF partitions, tokens on the free axis) so that
- every linear runs as matmul(lhsT=W_block, rhs=act) with W in natural layout,
- adaLN modulate scale/bias are per-partition tensor_scalar operands,
- LN token-stats come from an all-ones lhsT matmul (partition-broadcast sums).
The residual stream, stem and LN statistics run in float32r (full PE rate,
~1e-4 matmul rel err); the branch matmuls (qkv/attention/proj/mlp/cond) run
in bf16 to fit SBUF — their outputs are small gated residual updates.
The attention v-bias is folded into proj_b host-side (softmax rows sum to 1).
"""
import sys

sys.path.insert(0, "/opt/trn_rl_repo")

import numpy as np
import ml_dtypes

import concourse.bass as bass
import concourse.mybir as mybir
from concourse import bacc
from concourse.tile import TileContext
from concourse.bass_utils import run_bass_kernel_spmd

P = 128
B, C_IN, IMG, PS = 32, 4, 256, 16
GRID = IMG // PS          # 16
L = GRID * GRID           # 256 tokens per image
D = 1024
KT = D // P               # 8
NH, HD = 16, 64
NL = 6
NCORES = 8
IPC = B // NCORES         # 4 images per core
T = IPC * L               # 1024 tokens per core
NS = 2                    # token slices of 512
SL = T // NS              # 512
MAX_L = 10000.0
LN_EPS = 1e-5

F32 = mybir.dt.float32
F32R = mybir.dt.float32r
BF16 = mybir.dt.bfloat16
AF = mybir.ActivationFunctionType
ALU = mybir.AluOpType


def _bias_cols():
    """Column map for the packed per-partition bias tile [128, NB]."""
    cols = {}
    c = 0

    def add(name, n):
        nonlocal c
        cols[name] = c
        c += n

    add("eps", 1)
    add("xe", 8)
    add("tb1", 8)
    add("tb2", 8)
    add("fin", 16)
    for l in range(NL):
        add(f"qk{l}", 16)
        add(f"proj{l}", 8)
        add(f"b1{l}", 32)
        add(f"b2{l}", 8)
        add(f"cond{l}", 48)
    return cols, c


BCOLS, NB = _bias_cols()


# ---------------------------------------------------------------- device build
def _build():
    nc = bacc.Bacc("TRN2", target_bir_lowering=False, debug=False)

    def dram(name, shape, dt, out=False):
        return nc.dram_tensor(name, list(shape), dt,
                              kind="ExternalOutput" if out else "ExternalInput").ap()

    xT0 = dram("xT0", [D, T], F32R)
    te_d = dram("te", [D, IPC], F32R)
    yemb_d = dram("yemb", [D, IPC], F32R)
    xe_w = dram("xe_w", [8, P, KT, P], F32R)
    tw1 = dram("tw1", [8, P, KT, P], F32R)
    tw2 = dram("tw2", [8, P, KT, P], F32R)
    fin_w = dram("fin_w", [16, P, KT, P], F32R)
    qk_w = dram("qk_w", [NL, 16, P, KT, P], BF16)
    v_w = dram("v_w", [NL, KT, P, D], BF16)
    proj_w = dram("proj_w", [NL, 8, P, KT, P], BF16)
    m1_w = dram("m1_w", [NL, 32, P, KT, P], BF16)
    m2_w = dram("m2_w", [NL, 2, 8, P, 16, P], BF16)
    cond_w = dram("cond_w", [NL, 48, P, KT, P], BF16)
    bias_pp = dram("bias_pp", [P, NB], F32)
    onesr_d = dram("onesr", [P, P], F32R)
    onesb_d = dram("onesb", [P, P], BF16)
    outT = dram("outT", [D, T], F32, out=True)

    from contextlib import ExitStack
    with TileContext(nc) as tc, ExitStack() as ctx:
        cst = ctx.enter_context(tc.tile_pool(name="cst", bufs=1))
        pA = ctx.enter_context(tc.tile_pool(name="pA", bufs=1))
        pB = ctx.enter_context(tc.tile_pool(name="pB", bufs=1))
        pD = ctx.enter_context(tc.tile_pool(name="pD", bufs=1))
        wp = ctx.enter_context(tc.tile_pool(name="wp", bufs=2))
        ep = ctx.enter_context(tc.tile_pool(name="ep", bufs=2))
        st = ctx.enter_context(tc.tile_pool(name="st", bufs=1))
        psp = ctx.enter_context(tc.tile_pool(name="psp", bufs=4, space="PSUM"))
        psS = ctx.enter_context(tc.tile_pool(name="psS", bufs=2, space="PSUM"))
        psO = ctx.enter_context(tc.tile_pool(name="psO", bufs=1, space="PSUM"))

        ones_r = cst.tile([P, P], F32R)
        nc.sync.dma_start(ones_r, onesr_d)
        ones_b = cst.tile([P, P], BF16)
        nc.sync.dma_start(ones_b, onesb_d)
        bpp = cst.tile([P, NB], F32)
        nc.sync.dma_start(bpp, bias_pp)

        def bcol(name, mt=0):
            return bpp[:, BCOLS[name] + mt: BCOLS[name] + mt + 1]

        # ---- conditioning path (tiny, N=4) ----
        # cpk groups: 0=te 1=yemb 2=u1 3=cond  (all f32r)
        cpk = cst.tile([P, KT, 4, IPC], F32R)
        nc.sync.dma_start(cpk[:, :, 0], te_d.rearrange("(kt p) i -> p kt i", p=P))
        nc.sync.dma_start(cpk[:, :, 1], yemb_d.rearrange("(kt p) i -> p kt i", p=P))
        scond = cst.tile([P, KT, IPC], BF16)

        def small_fm(wdram, n_mt, act_grp, epilogue, wtag="w8", dt_=None):
            for mt in range(n_mt):
                wt = wp.tile([P, KT, P], dt_ or F32R, tag=wtag)
                nc.sync.dma_start(wt, wdram[mt])
                ps = psp.tile([P, IPC], F32, tag="ps512")
                for kt in range(KT):
                    nc.tensor.matmul(ps, wt[:, kt], act_grp(kt),
                                     start=kt == 0, stop=kt == KT - 1)
                epilogue(ps, mt)

        small_fm(tw1, 8, lambda kt: cpk[:, kt, 0],
                 lambda ps, mt: nc.scalar.activation(
                     cpk[:, mt, 2], ps, AF.Silu, bias=bcol("tb1", mt)))

        def temb_ep(ps, mt):
            tmp = ep.tile([P, SL], F32, tag="ptmp")
            nc.scalar.activation(tmp[:, :IPC], ps, AF.Identity, bias=bcol("tb2", mt))
            nc.vector.tensor_add(cpk[:, mt, 3], tmp[:, :IPC], cpk[:, mt, 1])
            nc.scalar.activation(scond[:, mt], cpk[:, mt, 3], AF.Silu)

        small_fm(tw2, 8, lambda kt: cpk[:, kt, 2], temb_ep)

        # mods for all layers + final, packed: [P, 6*48+16, IPC] f32
        mpk = cst.tile([P, NL * 48 + 16, IPC], F32)
        for l in range(NL):
            small_fm(cond_w[l], 48, lambda kt: scond[:, kt],
                     lambda ps, mt, l=l: nc.scalar.activation(
                         mpk[:, l * 48 + mt], ps, AF.Identity,
                         bias=bcol(f"cond{l}", mt)),
                     wtag="w8b", dt_=BF16)
        small_fm(fin_w, 16, lambda kt: cpk[:, kt, 3],
                 lambda ps, mt: nc.scalar.activation(
                     mpk[:, NL * 48 + mt], ps, AF.Identity, bias=bcol("fin", mt)))

        # ---- stem: h = patchify(x) @ x_embed_w + b (feature-major) ----
        hT = cst.tile([P, KT, T], F32R)
        for s in range(NS):
            xsb = pA.tile([P, KT, SL], F32R, tag="slotA")
            for kt in range(KT):
                nc.sync.dma_start(xsb[:, kt],
                                  xT0[kt * P:(kt + 1) * P, s * SL:(s + 1) * SL])
            for mt in range(KT):
                wt = wp.tile([P, KT, P], F32R, tag="w8")
                nc.sync.dma_start(wt, xe_w[mt])
                ps = psp.tile([P, SL], F32, tag="ps512")
                for kt in range(KT):
                    nc.tensor.matmul(ps, wt[:, kt], xsb[:, kt],
                                     start=kt == 0, stop=kt == KT - 1)
                nc.scalar.activation(hT[:, mt, s * SL:(s + 1) * SL], ps,
                                     AF.Identity, bias=bcol("xe", mt))

        def linear_fm(wdram, n_mt, act, epilogue, dt_=BF16, wtag="w8b"):
            # out[mt,s] = sum_kt W[kt,mt].T @ act[kt,s]
            for mt in range(n_mt):
                wt = wp.tile([P, KT, P], dt_, tag=wtag)
                nc.sync.dma_start(wt, wdram[mt])
                for s in range(NS):
                    ps = psp.tile([P, SL], F32, tag="ps512")
                    for kt in range(KT):
                        nc.tensor.matmul(ps, wt[:, kt],
                                         act[:, kt, s * SL:(s + 1) * SL],
                                         start=kt == 0, stop=kt == KT - 1)
                    epilogue(ps, mt, s)

        # ---- LN + modulate (feature-major; stats via all-ones matmul) ----
        def ln_mod(src, s_col, b_col, out_t=None, final=False):
            """out = (src - mu) * rstd * mods[s_col] + mods[b_col] per image.
            final=True: stream [P,SL] pieces straight to outT DRAM."""
            for s in range(NS):
                sc = slice(s * SL, (s + 1) * SL)
                sq = pA.tile([P, KT, SL], F32R, tag="slotA")
                for kt in range(KT):
                    nc.vector.tensor_mul(sq[:, kt], src[:, kt, sc], src[:, kt, sc])
                ps_sum = psp.tile([P, SL], F32, tag="ps512")
                for kt in range(KT):
                    nc.tensor.matmul(ps_sum, ones_r, src[:, kt, sc],
                                     start=kt == 0, stop=kt == KT - 1)
                ps_sq = psp.tile([P, SL], F32, tag="ps512")
                for kt in range(KT):
                    nc.tensor.matmul(ps_sq, ones_r, sq[:, kt],
                                     start=kt == 0, stop=kt == KT - 1)
                # stats pack: 0=mu 1=var 2=musq 3=rstd
                sp = st.tile([P, 4, SL], F32, tag="stats")
                nc.vector.tensor_scalar_mul(sp[:, 0], ps_sum, 1.0 / D)
                nc.vector.tensor_scalar_mul(sp[:, 1], ps_sq, 1.0 / D)
                nc.vector.tensor_mul(sp[:, 2], sp[:, 0], sp[:, 0])
                nc.vector.tensor_tensor(sp[:, 1], sp[:, 1], sp[:, 2], ALU.subtract)
                nc.scalar.activation(sp[:, 1], sp[:, 1], AF.Sqrt, bias=bcol("eps"))
                nc.vector.reciprocal(sp[:, 3], sp[:, 1])
                for kt in range(KT):
                    t_ = ep.tile([P, SL], F32, tag="lnt")
                    nc.vector.tensor_tensor(t_, src[:, kt, sc], sp[:, 0],
                                            ALU.subtract)
                    nc.vector.tensor_tensor(t_, t_, sp[:, 3], ALU.mult)
                    if final:
                        o = ep.tile([P, SL], F32, tag="ptmp", name="ofin")
                    else:
                        o = None
                    for i2 in range(2):
                        img = 2 * s + i2
                        i2c = slice(i2 * L, (i2 + 1) * L)
                        dst = o[:, i2c] if final else out_t[:, kt, img * L:(img + 1) * L]
                        nc.vector.tensor_scalar(
                            dst, t_[:, i2c],
                            mpk[:, s_col + kt, img:img + 1],
                            mpk[:, b_col + kt, img:img + 1],
                            ALU.mult, ALU.add)
                    if final:
                        nc.gpsimd.dma_start(outT[kt * P:(kt + 1) * P, sc], o)

        # ---- transformer layers ----
        for l in range(NL):
            mb = l * 48
            # attention branch
            a1 = pB.tile([P, KT, T], BF16, tag="slotB")
            ln_mod(hT, mb + 0, mb + 8, a1)

            qkT = pD.tile([P, 16, T], BF16, tag="slotD")
            linear_fm(qk_w[l], 16, a1, lambda ps, mt, s, l=l: nc.scalar.activation(
                qkT[:, mt, s * SL:(s + 1) * SL], ps, AF.Identity,
                bias=bcol(f"qk{l}", mt)))

            vtok = pA.tile([P, KT, D], BF16, tag="slotA")
            for nh in range(4):
                vw = wp.tile([P, KT, 256], BF16, tag="vw")
                nc.sync.dma_start(
                    vw, v_w[l][:, :, nh * 256:(nh + 1) * 256].rearrange(
                        "kt p n -> p kt n"))
                for vt in range(KT):
                    ps = psp.tile([P, 256], F32, tag="ps512")
                    for kt in range(KT):
                        nc.tensor.matmul(ps, a1[:, kt, vt * P:(vt + 1) * P],
                                         vw[:, kt], start=kt == 0,
                                         stop=kt == KT - 1)
                    nc.scalar.activation(vtok[:, vt, nh * 256:(nh + 1) * 256],
                                         ps, AF.Copy)

            attnT = pB.tile([P, KT, T], BF16, tag="slotB")
            for img in range(IPC):
                ic = slice(img * L, (img + 1) * L)
                for h in range(NH):
                    off = 64 * (h % 2)
                    pt = h // 2
                    qs = qkT[off:off + 64, pt, ic]
                    expS = ep.tile([P, 2, L], BF16, tag="expS")
                    for jt in range(2):
                        pss = psS.tile([P, L], F32, tag="psS")
                        j0 = img * L + jt * P
                        ks = qkT[off:off + 64, 8 + pt, j0:j0 + P]
                        nc.tensor.matmul(pss, ks, qs, start=True, stop=True)
                        nc.scalar.activation(expS[:, jt], pss, AF.Exp,
                                             scale=float(HD) ** -0.5)
                    psd = psO.tile([P, L], F32, tag="psD")
                    for jt in range(2):
                        nc.tensor.matmul(psd, ones_b, expS[:, jt],
                                         start=jt == 0, stop=jt == 1)
                    psv = psO.tile([64, L], F32, tag="psV")
                    for jt in range(2):
                        nc.tensor.matmul(psv,
                                         vtok[:, img * 2 + jt, h * 64:(h + 1) * 64],
                                         expS[:, jt], start=jt == 0, stop=jt == 1)
                    rec = ep.tile([64, L], F32, tag="rec")
                    nc.vector.reciprocal(rec, psd[0:64, :])
                    nc.vector.tensor_tensor(attnT[off:off + 64, pt, ic],
                                            psv, rec, ALU.mult)

            def proj_ep(ps, mt, s, l=l, mb=mb):
                sc = slice(s * SL, (s + 1) * SL)
                tmp = ep.tile([P, SL], F32, tag="ptmp")
                nc.scalar.activation(tmp, ps, AF.Identity, bias=bcol(f"proj{l}", mt))
                for i2 in range(2):
                    img = 2 * s + i2
                    nc.vector.tensor_scalar_mul(
                        tmp[:, i2 * L:(i2 + 1) * L], tmp[:, i2 * L:(i2 + 1) * L],
                        mpk[:, mb + 16 + mt, img:img + 1])
                nc.vector.tensor_tensor(hT[:, mt, sc], hT[:, mt, sc], tmp, ALU.add)

            linear_fm(proj_w[l], 8, attnT, proj_ep)

            # mlp branch
            a2 = pB.tile([P, KT, T], BF16, tag="slotB")
            ln_mod(hT, mb + 24, mb + 32, a2)
            macc = pA.tile([P, KT, T], BF16, tag="slotA")
            h1 = pD.tile([P, 16, T], BF16, tag="slotD")
            for hf in range(2):
                linear_fm(m1_w[l][hf * 16:(hf + 1) * 16], 16, a2,
                          lambda ps, mt, s, hf=hf, l=l: nc.scalar.activation(
                              h1[:, mt, s * SL:(s + 1) * SL], ps, AF.Silu,
                              bias=bcol(f"b1{l}", hf * 16 + mt)))
                for mt in range(KT):
                    w2t = wp.tile([P, 16, P], BF16, tag="w16")
                    nc.sync.dma_start(w2t, m2_w[l, hf, mt])
                    for s in range(NS):
                        sc = slice(s * SL, (s + 1) * SL)
                        ps = psp.tile([P, SL], F32, tag="ps512")
                        for kt2 in range(16):
                            nc.tensor.matmul(ps, w2t[:, kt2], h1[:, kt2, sc],
                                             start=kt2 == 0, stop=kt2 == 15)
                        if hf == 0:
                            nc.vector.tensor_copy(macc[:, mt, sc], ps)
                        else:
                            tmp = ep.tile([P, SL], F32, tag="ptmp")
                            nc.vector.tensor_tensor(tmp, macc[:, mt, sc], ps,
                                                    ALU.add)
                            for i2 in range(2):
                                img = 2 * s + i2
                                i2c = slice(i2 * L, (i2 + 1) * L)
                                nc.vector.tensor_scalar(
                                    tmp[:, i2c], tmp[:, i2c],
                                    bcol(f"b2{l}", mt),
                                    mpk[:, mb + 40 + mt, img:img + 1],
                                    ALU.add, ALU.mult)
                            nc.vector.tensor_tensor(hT[:, mt, sc],
                                                    hT[:, mt, sc], tmp, ALU.add)

        # ---- final LN + modulate -> outT ----
        ln_mod(hT, NL * 48 + 0, NL * 48 + 8, final=True)

    nc.compile()
    return nc


_NC_CACHE = None


def _get_nc():
    global _NC_CACHE
    if _NC_CACHE is None:
        _NC_CACHE = _build()
    return _NC_CACHE


# ---------------------------------------------------------------- host side
def _blocks(w, dtype=np.float32):
    """[K, M] -> [MT, 128(k), KT, 128(m)] so each mt slice is one DMA."""
    K, M = w.shape
    return np.ascontiguousarray(
        w.reshape(K // P, P, M // P, P).transpose(2, 1, 0, 3)).astype(dtype)


def _pack_biases(inp, proj_b_eff):
    bp = np.zeros((P, NB), np.float32)

    def put(name, b, bake1=()):
        b = np.asarray(b, np.float32).copy()
        for lo, hi in bake1:
            b[lo:hi] += 1.0
        n = b.shape[0] // P
        bp[:, BCOLS[name]:BCOLS[name] + n] = b.reshape(n, P).T

    bp[:, BCOLS["eps"]] = LN_EPS
    put("xe", inp["x_embed_b"])
    put("tb1", inp["t_b1"])
    put("tb2", inp["t_b2"])
    put("fin", inp["final_b"], bake1=[(0, D)])
    for l in range(NL):
        put(f"qk{l}", inp["qkv_b"][l][:2 * D])
        put(f"proj{l}", proj_b_eff[l])
        put(f"b1{l}", inp["mlp_b1"][l])
        put(f"b2{l}", inp["mlp_b2"][l])
        put(f"cond{l}", inp["cond_b"][l], bake1=[(0, D), (3 * D, 4 * D)])
    return bp


def prep_in_maps(inputs):
    inp = {k: np.asarray(v) for k, v in inputs.items()}
    bf16 = ml_dtypes.bfloat16

    x = inp["x"].astype(np.float32)
    patch = x.reshape(B, C_IN, GRID, PS, GRID, PS).transpose(
        0, 2, 4, 1, 3, 5).reshape(B, L, C_IN * PS * PS)

    t = inp["t"].astype(np.float32)
    angles = MAX_L ** (-(np.arange(0, D, 2, dtype=np.float32) / D))
    te = t[:, None] * angles[None, :]
    te = np.concatenate([np.sin(te), np.cos(te)], axis=-1).astype(np.float32)
    yemb = inp["y_table"][inp["y"]].astype(np.float32)

    # fold v-bias into proj bias: softmax rows sum to 1 -> attn_out += v_bias
    vb = inp["qkv_b"][:, 2 * D:].astype(np.float32)                  # [NL, D]
    proj_b_eff = inp["proj_b"].astype(np.float32) + np.einsum(
        "ld,ldm->lm", vb, inp["proj_w"].astype(np.float32))

    shared = {
        "xe_w": _blocks(inp["x_embed_w"]),
        "tw1": _blocks(inp["t_w1"]),
        "tw2": _blocks(inp["t_w2"]),
        "fin_w": _blocks(inp["final_w"]),
        "qk_w": np.stack([_blocks(inp["qkv_w"][l][:, :2 * D], bf16)
                          for l in range(NL)]),
        "v_w": np.ascontiguousarray(inp["qkv_w"][:, :, 2 * D:]).reshape(
            NL, KT, P, D).astype(bf16),
        "proj_w": np.stack([_blocks(inp["proj_w"][l], bf16) for l in range(NL)]),
        "m1_w": np.stack([_blocks(inp["mlp_w1"][l], bf16) for l in range(NL)]),
        "m2_w": np.stack([
            inp["mlp_w2"][l].reshape(2, 16, P, KT, P).transpose(0, 3, 2, 1, 4)
            for l in range(NL)]).astype(bf16),
        "cond_w": np.stack([_blocks(inp["cond_w"][l], bf16) for l in range(NL)]),
        "bias_pp": _pack_biases(inp, proj_b_eff),
        "onesr": np.ones((P, P), np.float32),
        "onesb": np.ones((P, P), bf16),
    }
    in_maps = []
    for c in range(NCORES):
        sl = slice(c * IPC, (c + 1) * IPC)
        m = dict(shared)
        m["xT0"] = np.ascontiguousarray(
            patch[sl].reshape(T, D).T, dtype=np.float32)
        m["te"] = np.ascontiguousarray(te[sl].T)
        m["yemb"] = np.ascontiguousarray(yemb[sl].T)
        in_maps.append(m)
    return in_maps


def gather_output(results):
    outs = []
    for c in range(NCORES):
        oT = results[c]["outT"]                           # [D, T]
        o = oT.T.reshape(IPC, GRID, GRID, C_IN, PS, PS)   # tokens -> patches
        outs.append(o.transpose(0, 3, 1, 4, 2, 5).reshape(IPC, C_IN, IMG, IMG))
    return np.concatenate(outs, axis=0).astype(np.float32)


def kernel(**inputs):
    nc = _get_nc()
    in_maps = prep_in_maps(inputs)
    res = run_bass_kernel_spmd(nc, in_maps, core_ids=list(range(NCORES)))
    return gather_output(res.results)


# revision 21
# speedup vs baseline: 582.2903x; 1.6551x over previous
"""DiT forward on 8 Trainium2 NeuronCores, data-parallel over batch.

Per-core program (4 images, T=1024 tokens): activations kept feature-major
(features on SBUF partitions, tokens on the free axis) so that
- every linear runs as matmul(lhsT=W_block, rhs=act) with W in natural layout,
- adaLN modulate scale/bias are per-partition tensor_scalar operands,
- LN token-stats come from an all-ones lhsT matmul (partition-broadcast sums).
The residual stream, stem and LN statistics run in float32r (full PE rate,
~1e-4 matmul rel err); the branch matmuls (qkv/attention/proj/mlp/cond) run
in bf16 to fit SBUF — their outputs are small gated residual updates.
The attention v-bias is folded into proj_b host-side (softmax rows sum to 1).
"""
import sys

sys.path.insert(0, "/opt/trn_rl_repo")

import numpy as np
import ml_dtypes

import concourse.bass as bass
import concourse.mybir as mybir
from concourse import bacc
from concourse.tile import TileContext
from concourse.bass_utils import run_bass_kernel_spmd

P = 128
B, C_IN, IMG, PS = 32, 4, 256, 16
GRID = IMG // PS          # 16
L = GRID * GRID           # 256 tokens per image
D = 1024
KT = D // P               # 8
NH, HD = 16, 64
NL = 6
NCORES = 8
IPC = B // NCORES         # 4 images per core
T = IPC * L               # 1024 tokens per core
NS = 2                    # token slices of 512
SL = T // NS              # 512
MAX_L = 10000.0
LN_EPS = 1e-5

F32 = mybir.dt.float32
F32R = mybir.dt.float32r
BF16 = mybir.dt.bfloat16
AF = mybir.ActivationFunctionType
ALU = mybir.AluOpType


def _bias_cols():
    """Column map for the packed per-partition bias tile [128, NB]."""
    cols = {}
    c = 0

    def add(name, n):
        nonlocal c
        cols[name] = c
        c += n

    add("eps", 1)
    add("xe", 8)
    add("tb1", 8)
    add("tb2", 8)
    add("fin", 16)
    for l in range(NL):
        add(f"qk{l}", 16)
        add(f"proj{l}", 8)
        add(f"b1{l}", 32)
        add(f"b2{l}", 8)
        add(f"cond{l}", 48)
    return cols, c


BCOLS, NB = _bias_cols()
PHASE_MARKS = []


# ---------------------------------------------------------------- device build
def _build():
    nc = bacc.Bacc("TRN2", target_bir_lowering=False, debug=False)

    def dram(name, shape, dt, out=False):
        return nc.dram_tensor(name, list(shape), dt,
                              kind="ExternalOutput" if out else "ExternalInput").ap()

    xT0 = dram("xT0", [D, T], F32R)
    te_d = dram("te", [D, IPC], F32R)
    yemb_d = dram("yemb", [D, IPC], F32R)
    xe_w = dram("xe_w", [8, P, KT, P], F32R)
    tw1 = dram("tw1", [8, P, KT, P], F32R)
    tw2 = dram("tw2", [8, P, KT, P], F32R)
    fin_w = dram("fin_w", [16, P, KT, P], F32R)
    qk_w = dram("qk_w", [NL, 16, P, KT, P], BF16)
    v_w = dram("v_w", [NL, KT, P, D], BF16)
    proj_w = dram("proj_w", [NL, 8, P, KT, P], BF16)
    m1_w = dram("m1_w", [NL, 32, P, KT, P], BF16)
    m2_w = dram("m2_w", [NL, 8, P, 32, P], BF16)
    cond_w = dram("cond_w", [NL, 48, P, KT, P], BF16)
    bias_pp = dram("bias_pp", [P, NB], F32)
    onesr_d = dram("onesr", [P, P], F32R)
    onesb_d = dram("onesb", [P, P], BF16)
    outT = dram("outT", [D, T], F32, out=True)

    def mark(name):
        nums = [int(n.split("-")[1]) for n in nc.inst_map
                if n.startswith("I-") and n.split("-")[1].isdigit()]
        PHASE_MARKS.append((name, max(nums) if nums else 0))

    from contextlib import ExitStack
    with TileContext(nc) as tc, ExitStack() as ctx:
        cst = ctx.enter_context(tc.tile_pool(name="cst", bufs=1))
        pA = ctx.enter_context(tc.tile_pool(name="pA", bufs=1))
        pB = ctx.enter_context(tc.tile_pool(name="pB", bufs=1))
        pD = ctx.enter_context(tc.tile_pool(name="pD", bufs=1))
        wp = ctx.enter_context(tc.tile_pool(name="wp", bufs=2))
        ep = ctx.enter_context(tc.tile_pool(name="ep", bufs=2))
        st = ctx.enter_context(tc.tile_pool(name="st", bufs=1))
        psp = ctx.enter_context(tc.tile_pool(name="psp", bufs=3, space="PSUM"))
        psS = ctx.enter_context(tc.tile_pool(name="psS", bufs=2, space="PSUM"))
        psD_p = ctx.enter_context(tc.tile_pool(name="psD_p", bufs=1, space="PSUM"))
        psV_p = ctx.enter_context(tc.tile_pool(name="psV_p", bufs=1, space="PSUM"))
        psC = ctx.enter_context(tc.tile_pool(name="psC", bufs=1, space="PSUM"))

        ones_r = cst.tile([P, P], F32R)
        nc.sync.dma_start(ones_r, onesr_d)
        ones_b = cst.tile([P, P], BF16)
        nc.sync.dma_start(ones_b, onesb_d)
        bpp = cst.tile([P, NB], F32)
        nc.sync.dma_start(bpp, bias_pp)

        def bcol(name, mt=0):
            return bpp[:, BCOLS[name] + mt: BCOLS[name] + mt + 1]

        mark("stem")
        # ---- stem: h = patchify(x) @ x_embed_w + b (feature-major) ----
        hT = cst.tile([P, KT, T], F32R)
        for s in range(NS):
            xsb = pA.tile([P, KT, SL], F32R, tag="slotA")
            for kt in range(KT):
                nc.sync.dma_start(xsb[:, kt],
                                  xT0[kt * P:(kt + 1) * P, s * SL:(s + 1) * SL])
            for mt in range(KT):
                wt = wp.tile([P, KT, P], F32R, tag="w8")
                nc.sync.dma_start(wt, xe_w[mt])
                ps = psp.tile([P, SL], F32, tag="ps512")
                for kt in range(KT):
                    nc.tensor.matmul(ps, wt[:, kt], xsb[:, kt],
                                     start=kt == 0, stop=kt == KT - 1)
                nc.scalar.activation(hT[:, mt, s * SL:(s + 1) * SL], ps,
                                     AF.Identity, bias=bcol("xe", mt))

        mark("cond")
        # ---- conditioning path (tiny, N=4) ----
        # cpk groups: 0=te 1=yemb 2=u1 3=cond  (all f32r)
        cpk = cst.tile([P, KT, 4, IPC], F32R)
        nc.sync.dma_start(cpk[:, :, 0], te_d.rearrange("(kt p) i -> p kt i", p=P))
        nc.sync.dma_start(cpk[:, :, 1], yemb_d.rearrange("(kt p) i -> p kt i", p=P))
        scond = cst.tile([P, KT, IPC], BF16)

        def small_fm_one(wdram, mt, act_grp, epilogue, wtag="w8", dt_=None,
                         dma_eng=None):
            wt = wp.tile([P, KT, P], dt_ or F32R, tag=wtag, name="wt_sm")
            (dma_eng or nc.sync).dma_start(wt, wdram[mt])
            ps = psC.tile([P, IPC], F32, tag="ps4", name="ps_sm")
            for kt in range(KT):
                nc.tensor.matmul(ps, wt[:, kt], act_grp(kt),
                                 start=kt == 0, stop=kt == KT - 1)
            epilogue(ps, mt)

        def small_fm(wdram, n_mt, act_grp, epilogue, wtag="w8", dt_=None,
                     dma_eng=None):
            for mt in range(n_mt):
                small_fm_one(wdram, mt, act_grp, epilogue, wtag, dt_, dma_eng)

        small_fm(tw1, 8, lambda kt: cpk[:, kt, 0],
                 lambda ps, mt: nc.scalar.activation(
                     cpk[:, mt, 2], ps, AF.Silu, bias=bcol("tb1", mt)))

        def temb_ep(ps, mt):
            tmp = ep.tile([P, SL], F32, tag="ptmp")
            nc.scalar.activation(tmp[:, :IPC], ps, AF.Identity, bias=bcol("tb2", mt))
            nc.vector.tensor_add(cpk[:, mt, 3], tmp[:, :IPC], cpk[:, mt, 1])
            nc.scalar.activation(scond[:, mt], cpk[:, mt, 3], AF.Silu)

        small_fm(tw2, 8, lambda kt: cpk[:, kt, 2], temb_ep)

        # mods for all layers + final, packed: [P, 6*48+16, IPC] f32.
        # Each layer's mods are emitted inside the previous layer's qk phase
        # so their (ACT-queue) DMAs overlap dense compute.
        mpk = cst.tile([P, NL * 48 + 16, IPC], F32)

        def mods_closures(l):
            def one(mt, l=l):
                small_fm_one(cond_w[l], mt, lambda kt: scond[:, kt],
                             lambda ps, mt, l=l: nc.vector.tensor_scalar_add(
                                 mpk[:, l * 48 + mt], ps,
                                 bcol(f"cond{l}", mt)),
                             wtag="wc", dt_=BF16, dma_eng=nc.scalar)
            return [lambda mt=mt: one(mt) for mt in range(48)]

        def fin_closures():
            def one(mt):
                small_fm_one(fin_w, mt, lambda kt: cpk[:, kt, 3],
                             lambda ps, mt: nc.vector.tensor_scalar_add(
                                 mpk[:, NL * 48 + mt], ps, bcol("fin", mt)),
                             wtag="wc", dma_eng=nc.scalar)
            return [lambda mt=mt: one(mt) for mt in range(16)]

        from collections import deque
        pending = deque()
        emit_mods_now = mods_closures(0)
        for fn in emit_mods_now:
            fn()

        def drain_pending(k=1):
            for _ in range(k):
                if pending:
                    pending.popleft()()

        def linear_fm(wdram, n_mt, act, epilogue, dt_=BF16, wtag="w8b",
                      s_list=None, hook=None):
            # out[mt,s] = sum_kt W[kt,mt].T @ act[kt,s]
            for mt in range(n_mt):
                wt = wp.tile([P, KT, P], dt_, tag=wtag)
                nc.sync.dma_start(wt, wdram[mt])
                for s in (s_list if s_list is not None else range(NS)):
                    ps = psp.tile([P, SL], F32, tag="ps512")
                    for kt in range(KT):
                        nc.tensor.matmul(ps, wt[:, kt],
                                         act[:, kt, s * SL:(s + 1) * SL],
                                         start=kt == 0, stop=kt == KT - 1)
                    epilogue(ps, mt, s)
                if hook is not None:
                    hook()

        # ---- LN + modulate (feature-major; stats via all-ones matmul) ----
        def ln_mod(src, s_col, b_col, out_t=None, final=False):
            """out = (src - mu) * rstd * mods[s_col] + mods[b_col] per image.
            final=True: stream [P,SL] pieces straight to outT DRAM."""
            for s in range(NS):
                sc = slice(s * SL, (s + 1) * SL)
                sq = pA.tile([P, KT, SL], F32R, tag="slotA")
                for kt in range(KT):
                    eng = nc.vector if kt % 2 == 0 else nc.gpsimd
                    eng.tensor_mul(sq[:, kt], src[:, kt, sc], src[:, kt, sc])
                ps_sum = psp.tile([P, SL], F32, tag="ps512")
                for kt in range(KT):
                    nc.tensor.matmul(ps_sum, ones_r, src[:, kt, sc],
                                     start=kt == 0, stop=kt == KT - 1)
                ps_sq = psp.tile([P, SL], F32, tag="ps512")
                for kt in range(KT):
                    nc.tensor.matmul(ps_sq, ones_r, sq[:, kt],
                                     start=kt == 0, stop=kt == KT - 1)
                # stats pack: 0=mu 1=var 2=musq 3=rstd
                sp = st.tile([P, 4, SL], F32, tag="stats")
                nc.vector.tensor_scalar_mul(sp[:, 0], ps_sum, 1.0 / D)
                nc.vector.tensor_scalar_mul(sp[:, 1], ps_sq, 1.0 / D)
                nc.vector.tensor_mul(sp[:, 2], sp[:, 0], sp[:, 0])
                nc.vector.tensor_tensor(sp[:, 1], sp[:, 1], sp[:, 2], ALU.subtract)
                nc.scalar.activation(sp[:, 1], sp[:, 1], AF.Sqrt, bias=bcol("eps"))
                nc.vector.reciprocal(sp[:, 3], sp[:, 1])
                for kt in range(KT):
                    eng = nc.vector if kt % 2 == 0 else nc.gpsimd
                    t_ = ep.tile([P, SL], F32, tag="lnt")
                    eng.tensor_tensor(t_, src[:, kt, sc], sp[:, 0],
                                      ALU.subtract)
                    eng.tensor_tensor(t_, t_, sp[:, 3], ALU.mult)
                    if final:
                        o = ep.tile([P, SL], F32, tag="ptmp", name="ofin")
                    else:
                        o = None
                    for i2 in range(2):
                        img = 2 * s + i2
                        i2c = slice(i2 * L, (i2 + 1) * L)
                        dst = o[:, i2c] if final else out_t[:, kt, img * L:(img + 1) * L]
                        eng.tensor_scalar(
                            dst, t_[:, i2c],
                            mpk[:, s_col + kt, img:img + 1],
                            mpk[:, b_col + kt, img:img + 1],
                            ALU.mult, ALU.add)
                    if final:
                        nc.gpsimd.dma_start(outT[kt * P:(kt + 1) * P, sc], o)

        # ---- transformer layers ----
        for l in range(NL):
            mb = l * 48
            # attention branch
            mark(f"L{l}.ln1")
            a1 = pB.tile([P, KT, T], BF16, tag="slotB")
            ln_mod(hT, mb + 0, mb + 8, a1)

            mark(f"L{l}.qk")
            qkT = pD.tile([P, 16, T], BF16, tag="slotD")
            linear_fm(qk_w[l], 16, a1, lambda ps, mt, s, l=l: nc.scalar.activation(
                qkT[:, mt, s * SL:(s + 1) * SL], ps, AF.Identity,
                bias=bcol(f"qk{l}", mt)))

            if l + 1 < NL:
                pending.extend(mods_closures(l + 1))
            else:
                pending.extend(fin_closures())

            mark(f"L{l}.v")
            vtok = pA.tile([P, KT, D], BF16, tag="slotA")
            for nh in range(4):
                vw = wp.tile([P, KT, 256], BF16, tag="vw")
                nc.scalar.dma_start(
                    vw, v_w[l][:, :, nh * 256:(nh + 1) * 256].rearrange(
                        "kt p n -> p kt n"))
                for vt in range(KT):
                    ps = psp.tile([P, 256], F32, tag="ps512")
                    for kt in range(KT):
                        nc.tensor.matmul(ps, a1[:, kt, vt * P:(vt + 1) * P],
                                         vw[:, kt], start=kt == 0,
                                         stop=kt == KT - 1)
                    nc.scalar.activation(vtok[:, vt, nh * 256:(nh + 1) * 256],
                                         ps, AF.Copy)

            mark(f"L{l}.attn")
            # head PAIRS (2k, 2k+1) share psum partition halves via
            # tile_position col-split -> full-width [128,L] recip/mult.
            attnT = pB.tile([P, KT, T], BF16, tag="slotB")
            pairs = [(img, k) for img in range(IPC) for k in range(NH // 2)]
            expS_live = {}

            def attn_s(i):
                img, k = pairs[i]
                # expS holds both heads of the pair: [:, hh, jt, :]
                expS = ep.tile([P, 2, 2, L], BF16, tag="expS", name=f"expS{i}")
                for hh in range(2):
                    off = 64 * hh
                    qs = qkT[off:off + 64, k, img * L:(img + 1) * L]
                    for jt in range(2):
                        pss = psS.tile([P, L], F32, tag="psS")
                        j0 = img * L + jt * P
                        ks = qkT[off:off + 64, 8 + k, j0:j0 + P]
                        nc.tensor.matmul(pss, ks, qs, start=True, stop=True)
                        nc.scalar.activation(expS[:, hh, jt], pss, AF.Exp,
                                             scale=float(HD) ** -0.5)
                expS_live[i] = expS

            def attn_av(i):
                img, k = pairs[i]
                ic = slice(img * L, (img + 1) * L)
                expS = expS_live.pop(i)
                psd = psD_p.tile([P, L], F32, tag="psD")
                psv = psV_p.tile([P, L], F32, tag="psV")
                for hh in range(2):
                    h = 2 * k + hh
                    tp = (0, 64 * hh)
                    for jt in range(2):
                        nc.tensor.matmul(psd[64 * hh:64 * hh + 64, :],
                                         ones_b[:, 0:64], expS[:, hh, jt],
                                         start=jt == 0, stop=jt == 1,
                                         tile_position=tp)
                        nc.tensor.matmul(psv[64 * hh:64 * hh + 64, :],
                                         vtok[:, img * 2 + jt, h * 64:(h + 1) * 64],
                                         expS[:, hh, jt],
                                         start=jt == 0, stop=jt == 1,
                                         tile_position=tp)
                rec = ep.tile([P, L], F32, tag="rec")
                nc.vector.reciprocal(rec, psd)
                nc.vector.tensor_tensor(attnT[:, k, ic], psv, rec, ALU.mult)

            attn_s(0)
            for i in range(len(pairs)):
                if i + 1 < len(pairs):
                    attn_s(i + 1)
                attn_av(i)

            def proj_ep(ps, mt, s, l=l, mb=mb):
                sc = slice(s * SL, (s + 1) * SL)
                tmp = ep.tile([P, SL], F32, tag="ptmp")
                nc.scalar.activation(tmp, ps, AF.Identity, bias=bcol(f"proj{l}", mt))
                for i2 in range(2):
                    img = 2 * s + i2
                    nc.vector.tensor_scalar_mul(
                        tmp[:, i2 * L:(i2 + 1) * L], tmp[:, i2 * L:(i2 + 1) * L],
                        mpk[:, mb + 16 + mt, img:img + 1])
                nc.vector.tensor_tensor(hT[:, mt, sc], hT[:, mt, sc], tmp, ALU.add)

            mark(f"L{l}.proj")
            linear_fm(proj_w[l], 8, attnT, proj_ep)

            # mlp branch
            mark(f"L{l}.ln2")
            a2 = pB.tile([P, KT, T], BF16, tag="slotB")
            ln_mod(hT, mb + 24, mb + 32, a2)
            mark(f"L{l}.mlp")
            # per token-slice: full 32-ktile dff in one psum chain (no macc)
            for s in range(NS):
                sc = slice(s * SL, (s + 1) * SL)
                h1 = pD.tile([P, 32, SL], BF16, tag="slotD", name=f"h1_{l}_{s}")
                linear_fm(m1_w[l], 32, a2,
                          lambda ps, mt, s2, l=l: nc.scalar.activation(
                              h1[:, mt], ps, AF.Silu,
                              bias=bcol(f"b1{l}", mt)),
                          s_list=[s], hook=drain_pending)
                for mt in range(KT):
                    w2t = wp.tile([P, 32, P], BF16, tag="w32")
                    nc.sync.dma_start(w2t, m2_w[l, mt])
                    ps = psp.tile([P, SL], F32, tag="ps512")
                    for kt2 in range(32):
                        nc.tensor.matmul(ps, w2t[:, kt2], h1[:, kt2],
                                         start=kt2 == 0, stop=kt2 == 31)
                    tmp = ep.tile([P, SL], F32, tag="ptmp")
                    for i2 in range(2):
                        img = 2 * s + i2
                        i2c = slice(i2 * L, (i2 + 1) * L)
                        nc.vector.tensor_scalar(
                            tmp[:, i2c], ps[:, i2c],
                            bcol(f"b2{l}", mt),
                            mpk[:, mb + 40 + mt, img:img + 1],
                            ALU.add, ALU.mult)
                    nc.vector.tensor_tensor(hT[:, mt, sc],
                                            hT[:, mt, sc], tmp, ALU.add)
                    drain_pending()

        mark("final")
        # ---- final LN + modulate -> outT ----
        ln_mod(hT, NL * 48 + 0, NL * 48 + 8, final=True)

    nc.compile()
    return nc


_NC_CACHE = None


def _get_nc():
    global _NC_CACHE
    if _NC_CACHE is None:
        _NC_CACHE = _build()
    return _NC_CACHE


# ---------------------------------------------------------------- host side
def _blocks(w, dtype=np.float32):
    """[K, M] -> [MT, 128(k), KT, 128(m)] so each mt slice is one DMA."""
    K, M = w.shape
    return np.ascontiguousarray(
        w.reshape(K // P, P, M // P, P).transpose(2, 1, 0, 3)).astype(dtype)


def _pack_biases(inp, proj_b_eff):
    bp = np.zeros((P, NB), np.float32)

    def put(name, b, bake1=()):
        b = np.asarray(b, np.float32).copy()
        for lo, hi in bake1:
            b[lo:hi] += 1.0
        n = b.shape[0] // P
        bp[:, BCOLS[name]:BCOLS[name] + n] = b.reshape(n, P).T

    bp[:, BCOLS["eps"]] = LN_EPS
    put("xe", inp["x_embed_b"])
    put("tb1", inp["t_b1"])
    put("tb2", inp["t_b2"])
    put("fin", inp["final_b"], bake1=[(0, D)])
    for l in range(NL):
        put(f"qk{l}", inp["qkv_b"][l][:2 * D])
        put(f"proj{l}", proj_b_eff[l])
        put(f"b1{l}", inp["mlp_b1"][l])
        put(f"b2{l}", inp["mlp_b2"][l])
        put(f"cond{l}", inp["cond_b"][l], bake1=[(0, D), (3 * D, 4 * D)])
    return bp


def prep_in_maps(inputs):
    inp = {k: np.asarray(v) for k, v in inputs.items()}
    bf16 = ml_dtypes.bfloat16

    x = inp["x"].astype(np.float32)
    patch = x.reshape(B, C_IN, GRID, PS, GRID, PS).transpose(
        0, 2, 4, 1, 3, 5).reshape(B, L, C_IN * PS * PS)

    t = inp["t"].astype(np.float32)
    angles = MAX_L ** (-(np.arange(0, D, 2, dtype=np.float32) / D))
    te = t[:, None] * angles[None, :]
    te = np.concatenate([np.sin(te), np.cos(te)], axis=-1).astype(np.float32)
    yemb = inp["y_table"][inp["y"]].astype(np.float32)

    # fold v-bias into proj bias: softmax rows sum to 1 -> attn_out += v_bias
    vb = inp["qkv_b"][:, 2 * D:].astype(np.float32)                  # [NL, D]
    proj_b_eff = inp["proj_b"].astype(np.float32) + np.einsum(
        "ld,ldm->lm", vb, inp["proj_w"].astype(np.float32))

    shared = {
        "xe_w": _blocks(inp["x_embed_w"]),
        "tw1": _blocks(inp["t_w1"]),
        "tw2": _blocks(inp["t_w2"]),
        "fin_w": _blocks(inp["final_w"]),
        "qk_w": np.stack([_blocks(inp["qkv_w"][l][:, :2 * D], bf16)
                          for l in range(NL)]),
        "v_w": np.ascontiguousarray(inp["qkv_w"][:, :, 2 * D:]).reshape(
            NL, KT, P, D).astype(bf16),
        "proj_w": np.stack([_blocks(inp["proj_w"][l], bf16) for l in range(NL)]),
        "m1_w": np.stack([_blocks(inp["mlp_w1"][l], bf16) for l in range(NL)]),
        "m2_w": np.stack([
            inp["mlp_w2"][l].reshape(32, P, KT, P).transpose(2, 1, 0, 3)
            for l in range(NL)]).astype(bf16),
        "cond_w": np.stack([_blocks(inp["cond_w"][l], bf16) for l in range(NL)]),
        "bias_pp": _pack_biases(inp, proj_b_eff),
        "onesr": np.ones((P, P), np.float32),
        "onesb": np.ones((P, P), bf16),
    }
    in_maps = []
    for c in range(NCORES):
        sl = slice(c * IPC, (c + 1) * IPC)
        m = dict(shared)
        m["xT0"] = np.ascontiguousarray(
            patch[sl].reshape(T, D).T, dtype=np.float32)
        m["te"] = np.ascontiguousarray(te[sl].T)
        m["yemb"] = np.ascontiguousarray(yemb[sl].T)
        in_maps.append(m)
    return in_maps


def gather_output(results):
    outs = []
    for c in range(NCORES):
        oT = results[c]["outT"]                           # [D, T]
        o = oT.T.reshape(IPC, GRID, GRID, C_IN, PS, PS)   # tokens -> patches
        outs.append(o.transpose(0, 3, 1, 4, 2, 5).reshape(IPC, C_IN, IMG, IMG))
    return np.concatenate(outs, axis=0).astype(np.float32)


def kernel(**inputs):
    nc = _get_nc()
    in_maps = prep_in_maps(inputs)
    res = run_bass_kernel_spmd(nc, in_maps, core_ids=list(range(NCORES)))
    return gather_output(res.results)


# revision 37
# speedup vs baseline: 8544.4565x; 14.6739x over previous
"""DiT forward on 8 Trainium2 NeuronCores, data-parallel over batch.

Per-core program (4 images, T=1024 tokens): activations kept feature-major
(features on SBUF partitions, tokens on the free axis) so that
- every linear runs as matmul(lhsT=W_block, rhs=act) with W in natural layout,
- adaLN modulate scale/bias are per-partition tensor_scalar operands,
- LN token-stats come from an all-ones lhsT matmul (partition-broadcast sums).
The residual stream, stem and LN statistics run in float32r (full PE rate,
~1e-4 matmul rel err); the branch matmuls (qkv/attention/proj/mlp/cond) run
in bf16 to fit SBUF — their outputs are small gated residual updates.
The attention v-bias is folded into proj_b host-side (softmax rows sum to 1).
"""
import sys

sys.path.insert(0, "/opt/trn_rl_repo")

import numpy as np
import ml_dtypes

import concourse.bass as bass
import concourse.mybir as mybir
from concourse import bacc
from concourse.tile import TileContext
from concourse.bass_utils import run_bass_kernel_spmd

P = 128
B, C_IN, IMG, PS = 32, 4, 256, 16
GRID = IMG // PS          # 16
L = GRID * GRID           # 256 tokens per image
D = 1024
KT = D // P               # 8
NH, HD = 16, 64
NL = 6
NCORES = 8
IPC = B // NCORES         # 4 images per core
T = IPC * L               # 1024 tokens per core
NS = 2                    # token slices of 512
SL = T // NS              # 512
MAX_L = 10000.0
LN_EPS = 1e-5

F32 = mybir.dt.float32
F32R = mybir.dt.float32r
BF16 = mybir.dt.bfloat16
AF = mybir.ActivationFunctionType
ALU = mybir.AluOpType


def _bias_cols():
    """Column map for the packed per-partition bias tile [128, NB]."""
    cols = {}
    c = 0

    def add(name, n):
        nonlocal c
        cols[name] = c
        c += n

    add("eps", 1)
    add("xe", 8)
    add("tb1", 8)
    add("tb2", 8)
    add("fin", 16)
    for l in range(NL):
        add(f"qk{l}", 16)
        add(f"proj{l}", 8)
        add(f"b1{l}", 32)
        add(f"b2{l}", 8)
        add(f"cond{l}", 48)
    return cols, c


BCOLS, NB = _bias_cols()
PHASE_MARKS = []


# ---------------------------------------------------------------- device build
def _build():
    nc = bacc.Bacc("TRN2", target_bir_lowering=False, debug=False)

    def dram(name, shape, dt, out=False):
        return nc.dram_tensor(name, list(shape), dt,
                              kind="ExternalOutput" if out else "ExternalInput").ap()

    xT0 = dram("xT0", [D, T], F32R)
    te_d = dram("te", [D, IPC], F32R)
    yemb_d = dram("yemb", [D, IPC], F32R)
    xe_w = dram("xe_w", [8, P, KT, P], F32R)
    tw1 = dram("tw1", [8, P, KT, P], F32R)
    tw2 = dram("tw2", [8, P, KT, P], F32R)
    fin_w = dram("fin_w", [16, P, KT, P], F32R)
    qk_w = dram("qk_w", [NL, 16, P, KT, P], BF16)
    v_w = dram("v_w", [NL, KT, P, D], BF16)
    proj_w = dram("proj_w", [NL, 8, P, KT, P], BF16)
    m1_w = dram("m1_w", [NL, 32, P, KT, P], BF16)
    m2_w = dram("m2_w", [NL, 8, P, 32, P], BF16)
    cond_w = dram("cond_w", [NL, 48, P, KT, P], BF16)
    bias_pp = dram("bias_pp", [P, NB], F32)
    onesr_d = dram("onesr", [P, P], F32R)
    onesb_d = dram("onesb", [P, P], BF16)
    outT = dram("outT", [D, T], F32, out=True)

    def mark(name):
        nums = [int(n.split("-")[1]) for n in nc.inst_map
                if n.startswith("I-") and n.split("-")[1].isdigit()]
        PHASE_MARKS.append((name, max(nums) if nums else 0))

    from contextlib import ExitStack
    with TileContext(nc) as tc, ExitStack() as ctx:
        cst = ctx.enter_context(tc.tile_pool(name="cst", bufs=1))
        pA = ctx.enter_context(tc.tile_pool(name="pA", bufs=1))
        pB = ctx.enter_context(tc.tile_pool(name="pB", bufs=1))
        pD = ctx.enter_context(tc.tile_pool(name="pD", bufs=1))
        wp = ctx.enter_context(tc.tile_pool(name="wp", bufs=3))
        ep = ctx.enter_context(tc.tile_pool(name="ep", bufs=3))
        ep4 = ctx.enter_context(tc.tile_pool(name="ep4", bufs=4))
        st = ctx.enter_context(tc.tile_pool(name="st", bufs=1))
        psp = ctx.enter_context(tc.tile_pool(name="psp", bufs=3, space="PSUM"))
        psS = ctx.enter_context(tc.tile_pool(name="psS", bufs=2, space="PSUM"))
        psD_p = ctx.enter_context(tc.tile_pool(name="psD_p", bufs=1, space="PSUM"))
        psV_p = ctx.enter_context(tc.tile_pool(name="psV_p", bufs=1, space="PSUM"))
        psC = ctx.enter_context(tc.tile_pool(name="psC", bufs=1, space="PSUM"))

        ones_r = cst.tile([P, P], F32R)
        nc.sync.dma_start(ones_r, onesr_d)
        ones_b = cst.tile([P, P], BF16)
        nc.sync.dma_start(ones_b, onesb_d)
        bpp = cst.tile([P, NB], F32)
        nc.sync.dma_start(bpp, bias_pp)

        def bcol(name, mt=0):
            return bpp[:, BCOLS[name] + mt: BCOLS[name] + mt + 1]

        mark("stem")
        # ---- stem: h = patchify(x) @ x_embed_w + b (feature-major) ----
        hT = cst.tile([P, KT, T], F32R)
        for s in range(NS):
            xsb = pA.tile([P, KT, SL], F32R, tag="slotA")
            for kt in range(KT):
                nc.sync.dma_start(xsb[:, kt],
                                  xT0[kt * P:(kt + 1) * P, s * SL:(s + 1) * SL])
            for mt in range(KT):
                wt = wp.tile([P, KT, P], F32R, tag="w8")
                nc.sync.dma_start(wt, xe_w[mt])
                ps = psp.tile([P, SL], F32, tag="ps512")
                for kt in range(KT):
                    nc.tensor.matmul(ps, wt[:, kt], xsb[:, kt],
                                     start=kt == 0, stop=kt == KT - 1)
                nc.scalar.activation(hT[:, mt, s * SL:(s + 1) * SL], ps,
                                     AF.Identity, bias=bcol("xe", mt))

        mark("cond")
        # ---- conditioning path (tiny, N=4) ----
        # cpk groups: 0=te 1=yemb 2=u1 3=cond  (all f32r)
        cpk = cst.tile([P, KT, 4, IPC], F32R)
        nc.sync.dma_start(cpk[:, :, 0], te_d.rearrange("(kt p) i -> p kt i", p=P))
        nc.sync.dma_start(cpk[:, :, 1], yemb_d.rearrange("(kt p) i -> p kt i", p=P))
        scond = cst.tile([P, KT, IPC], BF16)

        def small_fm_one(wdram, mt, act_grp, epilogue, wtag="w8", dt_=None,
                         dma_eng=None):
            wt = wp.tile([P, KT, P], dt_ or F32R, tag=wtag, name="wt_sm")
            (dma_eng or nc.sync).dma_start(wt, wdram[mt])
            ps = psC.tile([P, IPC], F32, tag="ps4", name="ps_sm")
            for kt in range(KT):
                nc.tensor.matmul(ps, wt[:, kt], act_grp(kt),
                                 start=kt == 0, stop=kt == KT - 1)
            epilogue(ps, mt)

        def small_fm(wdram, n_mt, act_grp, epilogue, wtag="w8", dt_=None,
                     dma_eng=None):
            for mt in range(n_mt):
                small_fm_one(wdram, mt, act_grp, epilogue, wtag, dt_, dma_eng)

        small_fm(tw1, 8, lambda kt: cpk[:, kt, 0],
                 lambda ps, mt: nc.scalar.activation(
                     cpk[:, mt, 2], ps, AF.Silu, bias=bcol("tb1", mt)))

        def temb_ep(ps, mt):
            tmp = ep4.tile([P, SL], F32, tag="ptmp")
            nc.scalar.activation(tmp[:, :IPC], ps, AF.Identity, bias=bcol("tb2", mt))
            nc.vector.tensor_add(cpk[:, mt, 3], tmp[:, :IPC], cpk[:, mt, 1])
            nc.scalar.activation(scond[:, mt], cpk[:, mt, 3], AF.Silu)

        small_fm(tw2, 8, lambda kt: cpk[:, kt, 2], temb_ep)

        # mods for all layers + final, packed: [P, 6*48+16, IPC] f32.
        # Each layer's mods are emitted inside the previous layer's qk phase
        # so their (ACT-queue) DMAs overlap dense compute.
        mpk = cst.tile([P, NL * 48 + 16, IPC], F32)

        def mods_closures(l):
            def one(mt, l=l):
                small_fm_one(cond_w[l], mt, lambda kt: scond[:, kt],
                             lambda ps, mt, l=l: nc.vector.tensor_scalar_add(
                                 mpk[:, l * 48 + mt], ps,
                                 bcol(f"cond{l}", mt)),
                             wtag="wc", dt_=BF16, dma_eng=nc.scalar)
            return [lambda mt=mt: one(mt) for mt in range(48)]

        def fin_closures():
            def one(mt):
                small_fm_one(fin_w, mt, lambda kt: cpk[:, kt, 3],
                             lambda ps, mt: nc.vector.tensor_scalar_add(
                                 mpk[:, NL * 48 + mt], ps, bcol("fin", mt)),
                             wtag="wc", dma_eng=nc.scalar)
            return [lambda mt=mt: one(mt) for mt in range(16)]

        from collections import deque
        pending = deque()
        emit_mods_now = mods_closures(0)
        for fn in emit_mods_now:
            fn()

        def drain_pending(k=1):
            for _ in range(k):
                if pending:
                    pending.popleft()()

        def linear_fm(wdram, n_mt, act, epilogue, dt_=BF16, wtag="w8b",
                      s_list=None, hook=None):
            # out[mt,s] = sum_kt W[kt,mt].T @ act[kt,s]
            for mt in range(n_mt):
                wt = wp.tile([P, KT, P], dt_, tag=wtag)
                nc.sync.dma_start(wt, wdram[mt])
                for s in (s_list if s_list is not None else range(NS)):
                    ps = psp.tile([P, SL], F32, tag="ps512")
                    for kt in range(KT):
                        nc.tensor.matmul(ps, wt[:, kt],
                                         act[:, kt, s * SL:(s + 1) * SL],
                                         start=kt == 0, stop=kt == KT - 1)
                    epilogue(ps, mt, s)
                if hook is not None:
                    hook()

        # ---- LN + modulate (feature-major; stats via all-ones matmul) ----
        def ln_mod(src, s_col, b_col, out_t=None, final=False):
            """out = (src - mu) * rstd * mods[s_col] + mods[b_col] per image.
            final=True: stream [P,SL] pieces straight to outT DRAM."""
            for s in range(NS):
                sc = slice(s * SL, (s + 1) * SL)
                sq = pA.tile([P, KT, SL], F32R, tag="slotA")
                for kt in range(KT):
                    eng = nc.vector if kt % 2 == 0 else nc.gpsimd
                    eng.tensor_mul(sq[:, kt], src[:, kt, sc], src[:, kt, sc])
                ps_sum = psp.tile([P, SL], F32, tag="ps512")
                for kt in range(KT):
                    nc.tensor.matmul(ps_sum, ones_r, src[:, kt, sc],
                                     start=kt == 0, stop=kt == KT - 1)
                ps_sq = psp.tile([P, SL], F32, tag="ps512")
                for kt in range(KT):
                    nc.tensor.matmul(ps_sq, ones_r, sq[:, kt],
                                     start=kt == 0, stop=kt == KT - 1)
                # stats pack: 0=mu 1=var 2=musq 3=rstd
                sp = st.tile([P, 4, SL], F32, tag="stats")
                nc.vector.tensor_scalar_mul(sp[:, 0], ps_sum, 1.0 / D)
                nc.vector.tensor_scalar_mul(sp[:, 1], ps_sq, 1.0 / D)
                nc.vector.tensor_mul(sp[:, 2], sp[:, 0], sp[:, 0])
                nc.vector.tensor_tensor(sp[:, 1], sp[:, 1], sp[:, 2], ALU.subtract)
                nc.scalar.activation(sp[:, 1], sp[:, 1], AF.Sqrt, bias=bcol("eps"))
                nc.vector.reciprocal(sp[:, 3], sp[:, 1])
                for kt in range(KT):
                    eng = nc.vector if kt % 2 == 0 else nc.gpsimd
                    t_ = ep4.tile([P, SL], F32, tag="lnt")
                    eng.tensor_tensor(t_, src[:, kt, sc], sp[:, 0],
                                      ALU.subtract)
                    eng.tensor_tensor(t_, t_, sp[:, 3], ALU.mult)
                    if final:
                        o = ep4.tile([P, SL], F32, tag="ptmp", name="ofin")
                    else:
                        o = None
                    for i2 in range(2):
                        img = 2 * s + i2
                        i2c = slice(i2 * L, (i2 + 1) * L)
                        dst = o[:, i2c] if final else out_t[:, kt, img * L:(img + 1) * L]
                        eng.tensor_scalar(
                            dst, t_[:, i2c],
                            mpk[:, s_col + kt, img:img + 1],
                            mpk[:, b_col + kt, img:img + 1],
                            ALU.mult, ALU.add)
                    if final:
                        nc.gpsimd.dma_start(outT[kt * P:(kt + 1) * P, sc], o)

        # ---- transformer layers ----
        for l in range(NL):
            mb = l * 48
            # attention branch
            mark(f"L{l}.ln1")
            a1 = pB.tile([P, KT, T], BF16, tag="slotB")
            ln_mod(hT, mb + 0, mb + 8, a1)

            mark(f"L{l}.qk")
            qkT = pD.tile([P, 16, T], BF16, tag="slotD")
            def qk_ep(ps, mt, s, l=l):
                dst = qkT[:, mt, s * SL:(s + 1) * SL]
                if s == 0:
                    nc.scalar.activation(dst, ps, AF.Identity,
                                         bias=bcol(f"qk{l}", mt))
                else:
                    nc.vector.tensor_scalar_add(dst, ps, bcol(f"qk{l}", mt))

            linear_fm(qk_w[l], 16, a1, qk_ep)

            if l + 1 < NL:
                pending.extend(mods_closures(l + 1))
            else:
                pending.extend(fin_closures())

            mark(f"L{l}.v")
            vtok = pA.tile([P, KT, D], BF16, tag="slotA")
            for nh in range(4):
                vw = wp.tile([P, KT, 256], BF16, tag="vw")
                nc.scalar.dma_start(
                    vw, v_w[l][:, :, nh * 256:(nh + 1) * 256].rearrange(
                        "kt p n -> p kt n"))
                for vt in range(KT):
                    ps = psp.tile([P, 256], F32, tag="ps512")
                    for kt in range(KT):
                        nc.tensor.matmul(ps, a1[:, kt, vt * P:(vt + 1) * P],
                                         vw[:, kt], start=kt == 0,
                                         stop=kt == KT - 1)
                    nc.scalar.activation(vtok[:, vt, nh * 256:(nh + 1) * 256],
                                         ps, AF.Copy)

            mark(f"L{l}.attn")
            # head PAIRS (2k, 2k+1) share psum partition halves via
            # tile_position col-split -> full-width [128,L] recip/mult.
            attnT = pB.tile([P, KT, T], BF16, tag="slotB")
            pairs = [(img, k) for img in range(IPC) for k in range(NH // 2)]
            expS_live = {}

            def attn_s(i):
                img, k = pairs[i]
                # expS holds both heads of the pair: [:, hh, jt, :]
                expS = ep.tile([P, 2, 2, L], BF16, tag="expS", name=f"expS{i}")
                for hh in range(2):
                    off = 64 * hh
                    qs = qkT[off:off + 64, k, img * L:(img + 1) * L]
                    for jt in range(2):
                        pss = psS.tile([P, L], F32, tag="psS")
                        j0 = img * L + jt * P
                        ks = qkT[off:off + 64, 8 + k, j0:j0 + P]
                        nc.tensor.matmul(pss, ks, qs, start=True, stop=True)
                        nc.scalar.activation(expS[:, hh, jt], pss, AF.Exp,
                                             scale=float(HD) ** -0.5)
                expS_live[i] = expS

            def attn_av(i):
                img, k = pairs[i]
                ic = slice(img * L, (img + 1) * L)
                expS = expS_live.pop(i)
                psd = psD_p.tile([P, L], F32, tag="psD")
                psv = psV_p.tile([P, L], F32, tag="psV")
                for hh in range(2):
                    h = 2 * k + hh
                    tp = (0, 64 * hh)
                    for jt in range(2):
                        nc.tensor.matmul(psd[64 * hh:64 * hh + 64, :],
                                         ones_b[:, 0:64], expS[:, hh, jt],
                                         start=jt == 0, stop=jt == 1,
                                         tile_position=tp)
                        nc.tensor.matmul(psv[64 * hh:64 * hh + 64, :],
                                         vtok[:, img * 2 + jt, h * 64:(h + 1) * 64],
                                         expS[:, hh, jt],
                                         start=jt == 0, stop=jt == 1,
                                         tile_position=tp)
                rec = ep.tile([P, L], F32, tag="rec")
                nc.vector.reciprocal(rec, psd)
                nc.vector.tensor_tensor(attnT[:, k, ic], psv, rec, ALU.mult)

            attn_s(0)
            for i in range(len(pairs)):
                if i + 1 < len(pairs):
                    attn_s(i + 1)
                attn_av(i)

            def proj_ep(ps, mt, s, l=l, mb=mb):
                sc = slice(s * SL, (s + 1) * SL)
                tmp = ep4.tile([P, SL], F32, tag="ptmp")
                nc.scalar.activation(tmp, ps, AF.Identity, bias=bcol(f"proj{l}", mt))
                for i2 in range(2):
                    img = 2 * s + i2
                    nc.vector.tensor_scalar_mul(
                        tmp[:, i2 * L:(i2 + 1) * L], tmp[:, i2 * L:(i2 + 1) * L],
                        mpk[:, mb + 16 + mt, img:img + 1])
                nc.vector.tensor_tensor(hT[:, mt, sc], hT[:, mt, sc], tmp, ALU.add)

            mark(f"L{l}.proj")
            linear_fm(proj_w[l], 8, attnT, proj_ep)

            # mlp branch
            mark(f"L{l}.ln2")
            a2 = pB.tile([P, KT, T], BF16, tag="slotB")
            ln_mod(hT, mb + 24, mb + 32, a2)
            mark(f"L{l}.mlp")
            # per token-slice: full 32-ktile dff in one psum chain (no macc)
            for s in range(NS):
                sc = slice(s * SL, (s + 1) * SL)
                h1 = pD.tile([P, 32, SL], BF16, tag="slotD", name=f"h1_{l}_{s}")
                linear_fm(m1_w[l], 32, a2,
                          lambda ps, mt, s2, l=l: nc.scalar.activation(
                              h1[:, mt], ps, AF.Silu,
                              bias=bcol(f"b1{l}", mt)),
                          s_list=[s], hook=drain_pending)
                for mt in range(KT):
                    w2t = wp.tile([P, 32, P], BF16, tag="w32")
                    nc.sync.dma_start(w2t, m2_w[l, mt])
                    ps = psp.tile([P, SL], F32, tag="ps512")
                    for kt2 in range(32):
                        nc.tensor.matmul(ps, w2t[:, kt2], h1[:, kt2],
                                         start=kt2 == 0, stop=kt2 == 31)
                    tmp = ep4.tile([P, SL], F32, tag="ptmp")
                    for i2 in range(2):
                        img = 2 * s + i2
                        i2c = slice(i2 * L, (i2 + 1) * L)
                        nc.vector.tensor_scalar(
                            tmp[:, i2c], ps[:, i2c],
                            bcol(f"b2{l}", mt),
                            mpk[:, mb + 40 + mt, img:img + 1],
                            ALU.add, ALU.mult)
                    nc.vector.tensor_tensor(hT[:, mt, sc],
                                            hT[:, mt, sc], tmp, ALU.add)
                    drain_pending()

        mark("final")
        # ---- final LN + modulate -> outT ----
        ln_mod(hT, NL * 48 + 0, NL * 48 + 8, final=True)

    nc.compile()
    return nc


_NC_CACHE = None


def _get_nc():
    global _NC_CACHE
    if _NC_CACHE is None:
        _NC_CACHE = _build()
    return _NC_CACHE


# ---------------------------------------------------------------- host side
def _blocks(w, dtype=np.float32):
    """[K, M] -> [MT, 128(k), KT, 128(m)] so each mt slice is one DMA."""
    K, M = w.shape
    return np.ascontiguousarray(
        w.reshape(K // P, P, M // P, P).transpose(2, 1, 0, 3)).astype(dtype)


def _pack_biases(inp, proj_b_eff):
    bp = np.zeros((P, NB), np.float32)

    def put(name, b, bake1=()):
        b = np.asarray(b, np.float32).copy()
        for lo, hi in bake1:
            b[lo:hi] += 1.0
        n = b.shape[0] // P
        bp[:, BCOLS[name]:BCOLS[name] + n] = b.reshape(n, P).T

    bp[:, BCOLS["eps"]] = LN_EPS
    put("xe", inp["x_embed_b"])
    put("tb1", inp["t_b1"])
    put("tb2", inp["t_b2"])
    put("fin", inp["final_b"], bake1=[(0, D)])
    for l in range(NL):
        put(f"qk{l}", inp["qkv_b"][l][:2 * D])
        put(f"proj{l}", proj_b_eff[l])
        put(f"b1{l}", inp["mlp_b1"][l])
        put(f"b2{l}", inp["mlp_b2"][l])
        put(f"cond{l}", inp["cond_b"][l], bake1=[(0, D), (3 * D, 4 * D)])
    return bp


def prep_in_maps(inputs):
    inp = {k: np.asarray(v) for k, v in inputs.items()}
    bf16 = ml_dtypes.bfloat16

    x = inp["x"].astype(np.float32)
    patch = x.reshape(B, C_IN, GRID, PS, GRID, PS).transpose(
        0, 2, 4, 1, 3, 5).reshape(B, L, C_IN * PS * PS)

    t = inp["t"].astype(np.float32)
    angles = MAX_L ** (-(np.arange(0, D, 2, dtype=np.float32) / D))
    te = t[:, None] * angles[None, :]
    te = np.concatenate([np.sin(te), np.cos(te)], axis=-1).astype(np.float32)
    yemb = inp["y_table"][inp["y"]].astype(np.float32)

    # fold v-bias into proj bias: softmax rows sum to 1 -> attn_out += v_bias
    vb = inp["qkv_b"][:, 2 * D:].astype(np.float32)                  # [NL, D]
    proj_b_eff = inp["proj_b"].astype(np.float32) + np.einsum(
        "ld,ldm->lm", vb, inp["proj_w"].astype(np.float32))

    shared = {
        "xe_w": _blocks(inp["x_embed_w"]),
        "tw1": _blocks(inp["t_w1"]),
        "tw2": _blocks(inp["t_w2"]),
        "fin_w": _blocks(inp["final_w"]),
        "qk_w": np.stack([_blocks(inp["qkv_w"][l][:, :2 * D], bf16)
                          for l in range(NL)]),
        "v_w": np.ascontiguousarray(inp["qkv_w"][:, :, 2 * D:]).reshape(
            NL, KT, P, D).astype(bf16),
        "proj_w": np.stack([_blocks(inp["proj_w"][l], bf16) for l in range(NL)]),
        "m1_w": np.stack([_blocks(inp["mlp_w1"][l], bf16) for l in range(NL)]),
        "m2_w": np.stack([
            inp["mlp_w2"][l].reshape(32, P, KT, P).transpose(2, 1, 0, 3)
            for l in range(NL)]).astype(bf16),
        "cond_w": np.stack([_blocks(inp["cond_w"][l], bf16) for l in range(NL)]),
        "bias_pp": _pack_biases(inp, proj_b_eff),
        "onesr": np.ones((P, P), np.float32),
        "onesb": np.ones((P, P), bf16),
    }
    in_maps = []
    for c in range(NCORES):
        sl = slice(c * IPC, (c + 1) * IPC)
        m = dict(shared)
        m["xT0"] = np.ascontiguousarray(
            patch[sl].reshape(T, D).T, dtype=np.float32)
        m["te"] = np.ascontiguousarray(te[sl].T)
        m["yemb"] = np.ascontiguousarray(yemb[sl].T)
        in_maps.append(m)
    return in_maps


def gather_output(results):
    outs = []
    for c in range(NCORES):
        oT = results[c]["outT"]                           # [D, T]
        o = oT.T.reshape(IPC, GRID, GRID, C_IN, PS, PS)   # tokens -> patches
        outs.append(o.transpose(0, 3, 1, 4, 2, 5).reshape(IPC, C_IN, IMG, IMG))
    return np.concatenate(outs, axis=0).astype(np.float32)


def kernel(**inputs):
    nc = _get_nc()
    in_maps = prep_in_maps(inputs)
    res = run_bass_kernel_spmd(nc, in_maps, core_ids=list(range(NCORES)))
    return gather_output(res.results)
